# revision 10
# baseline (speedup 1.0000x reference)
import numpy as np

N_ROWS = 4096
D = 256
N_CORES = 8
ROWS_PER_CORE = N_ROWS // N_CORES  # 512
C_SLOTS = ROWS_PER_CORE // 128     # 4
FREE = C_SLOTS * D                 # 1024
HALF = FREE // 2                   # 512 (c-slots 0-1 vs 2-3)
K_SAFE = 48                        # Knuth TwoSum below this chain index, Fast2Sum after


def _build_nc(debug=False):
    from concourse import bass
    from concourse import mybir
    from contextlib import ExitStack

    f32 = mybir.dt.float32
    nc = bass.Bass()
    x_ext = nc.declare_dram_parameter("x", [128, FREE], f32, isOutput=False)
    part_ext = nc.declare_dram_parameter("partial", [128, 1], f32, isOutput=True)
    dbg_ext = {}
    if debug:
        for nm in ["s_out", "g_out", "d2_out", "dact", "dnew"]:
            dbg_ext[nm] = nc.declare_dram_parameter(nm, [128, C_SLOTS], f32, isOutput=True)

    with ExitStack() as ctx:
        dma_sem = ctx.enter_context(nc.semaphore("dma_sem"))
        v_sem = ctx.enter_context(nc.semaphore("v_sem"))
        a_sem = ctx.enter_context(nc.semaphore("a_sem"))

        def sb(name, shape):
            return ctx.enter_context(nc.sbuf_tensor(name, shape, f32))

        x_sb = sb("x_sb", [128, FREE])
        sq = sb("sq_sb", [128, FREE])
        lo = sb("lo_sb", [128, FREE])
        tmp1 = sb("tmp1", [128, FREE])
        tmp2 = sb("tmp2", [128, FREE])
        tmp3 = sb("tmp3", [128, FREE])
        wsum = sb("wsum", [128, C_SLOTS * 8])
        ssum = sb("ssum", [128, C_SLOTS])
        acc = sb("acc", [128, C_SLOTS])
        cs = sb("cs", [128, C_SLOTS])
        cbb = sb("cbb", [128, C_SLOTS])
        ct1 = sb("ct1", [128, C_SLOTS])
        ce1 = sb("ce1", [128, C_SLOTS])
        ce2 = sb("ce2", [128, C_SLOTS])
        ce = sb("ce", [128, C_SLOTS])
        ct = sb("ct", [128, C_SLOTS])
        d2c = sb("d2c", [128, C_SLOTS])
        dact_sb = sb("dact_sb", [128, C_SLOTS])
        dnew_sb = sb("dnew_sb", [128, C_SLOTS])
        part_sb = sb("part_sb", [128, 1])
        block = ctx.enter_context(nc.Block())

        @block.sync
        def _(sync):
            sync.dma_start(out=x_sb[:], in_=x_ext[:]).then_inc(dma_sem, 16)
            sync.wait_ge(v_sem, 2)
            sync.dma_start(out=part_ext[:], in_=part_sb[:]).then_inc(dma_sem, 16)
            n_out = 2
            if debug:
                for nm, src in [("s_out", ssum), ("g_out", acc), ("d2_out", d2c),
                                ("dact", dact_sb), ("dnew", dnew_sb)]:
                    sync.dma_start(out=dbg_ext[nm][:], in_=src[:]).then_inc(dma_sem, 16)
                    n_out += 1
            sync.wait_ge(dma_sem, 16 * n_out)

        @block.vector
        def _(v):
            v.wait_ge(dma_sem, 16)

            # Two independent half-programs (c-slots 0-1 vs 2-3) with strictly
            # alternating instructions: a DVE write is not visible to the
            # immediately-next DVE instruction, but is at distance >= 2.
            def half_prog(h):
                fb = h * HALF              # free-dim base into (128, FREE) tensors
                cb = h * (C_SLOTS // 2)    # slot base into (128, C_SLOTS) tensors
                nsl = C_SLOTS // 2         # 2 slots per half

                def big(t):  # (128, HALF) view
                    return t[:, fb:fb + HALF]

                def small(t):  # (128, 2) view
                    return t[:, cb:cb + nsl]

                yield v.tensor_mul(big(sq), big(x_sb), big(x_sb))

                # ---- s_i: 8 windows of 32 sequential adds, then sequential
                # combine of the 8 window sums (XLA-CPU reduce scheme) ----
                wv = wsum[:, cb * 8:(cb + nsl) * 8]  # (128,16)

                def sq_j(j):
                    return sq[:, fb + j:fb + HALF:32]  # (128,16)

                yield v.tensor_copy(wv, sq_j(0))
                for j in range(1, 32):
                    yield v.tensor_add(wv, wv, sq_j(j))
                sv = small(ssum)

                def w_w(w):
                    return wsum[:, cb * 8 + w:(cb + nsl) * 8:8]  # (128,2)

                yield v.tensor_copy(sv, w_w(0))
                for w in range(1, 8):
                    yield v.tensor_add(sv, sv, w_w(w))

                # ---- Dekker: lo_k = x^2 - fl(x^2) exactly ----
                yield v.tensor_scalar_mul(big(tmp1), big(x_sb), 4097.0)
                yield v.tensor_sub(big(tmp2), big(tmp1), big(x_sb))
                yield v.tensor_sub(big(tmp1), big(tmp1), big(tmp2))     # xh
                yield v.tensor_sub(big(tmp2), big(x_sb), big(tmp1))     # xl
                yield v.tensor_mul(big(tmp3), big(tmp1), big(tmp1))     # xh*xh
                yield v.tensor_sub(big(tmp3), big(tmp3), big(sq))
                yield v.tensor_mul(big(tmp1), big(tmp1), big(tmp2))     # xh*xl
                yield v.tensor_add(big(tmp1), big(tmp1), big(tmp1))     # 2*xh*xl
                yield v.tensor_add(big(tmp3), big(tmp3), big(tmp1))
                yield v.tensor_mul(big(tmp2), big(tmp2), big(tmp2))     # xl*xl
                yield v.tensor_add(big(lo), big(tmp3), big(tmp2))

                # ---- sequential FMA chain: acc = fma(x_k, x_k, acc) ----
                def hi_k(k):
                    return sq[:, fb + k:fb + HALF:D]  # (128,2)

                def lo_k(k):
                    return lo[:, fb + k:fb + HALF:D]

                av, csv, cbv = small(acc), small(cs), small(cbb)
                t1v, e1v, e2v = small(ct1), small(ce1), small(ce2)
                ev, tv = small(ce), small(ct)
                yield v.tensor_copy(av, hi_k(0))
                for k in range(1, D):
                    h = hi_k(k)
                    if k < K_SAFE:  # Knuth TwoSum
                        yield v.tensor_add(csv, av, h)
                        yield v.tensor_sub(cbv, csv, av)
                        yield v.tensor_sub(t1v, csv, cbv)
                        yield v.tensor_sub(e1v, av, t1v)
                        yield v.tensor_sub(e2v, h, cbv)
                        yield v.tensor_add(ev, e1v, e2v)
                    else:  # Fast2Sum
                        yield v.tensor_add(csv, av, h)
                        yield v.tensor_sub(cbv, csv, av)
                        yield v.tensor_sub(ev, h, cbv)
                    yield v.tensor_add(tv, ev, lo_k(k))
                    yield v.tensor_add(av, csv, tv)

                # ---- d2 = fl(2s - 2g), clip at 0 ----
                yield v.tensor_add(csv, sv, sv)
                yield v.tensor_add(cbv, av, av)
                yield v.tensor_sub(tv, csv, cbv)
                yield v.tensor_scalar_max(small(d2c), tv, 0.0)

            g0, g1 = half_prog(0), half_prog(1)
            while True:
                done = 0
                for g in (g0, g1):
                    try:
                        next(g)
                    except StopIteration:
                        done += 1
                if done == 2:
                    break
            # spacer guaranteeing d2c committed before cross-engine read
            v.tensor_copy(tmp1[:, 0:HALF], sq[:, 0:HALF]).then_inc(v_sem)

            # ---- Newton refine: d = 0.5*(y + d2 * recip(y)) ----
            v.wait_ge(a_sem, 1)

            def newton(h):
                cb = h * (C_SLOTS // 2)
                nsl = C_SLOTS // 2

                def small(t):
                    return t[:, cb:cb + nsl]

                yv, t1v, r1v = small(dact_sb), small(ct1), small(ce1)
                yield v.tensor_scalar_max(t1v, yv, 1e-30)
                yield v.reciprocal(r1v, t1v)
                yield v.tensor_mul(r1v, r1v, small(d2c))
                yield v.tensor_add(small(ce2), yv, r1v)
                yield v.tensor_scalar_mul(small(dnew_sb), small(ce2), 0.5)

            g0, g1 = newton(0), newton(1)
            while True:
                done = 0
                for g in (g0, g1):
                    try:
                        next(g)
                    except StopIteration:
                        done += 1
                if done == 2:
                    break
            # spacer before reading dnew in the reduce
            v.tensor_copy(tmp2[:, 0:HALF], sq[:, 0:HALF])
            v.tensor_reduce(part_sb[:], dnew_sb[:], mybir.AxisListType.X,
                            mybir.AluOpType.add).then_inc(v_sem)

        @block.scalar
        def _(s):
            s.wait_ge(v_sem, 1)
            s.sqrt(dact_sb[:], d2c[:]).then_inc(a_sem)

    return nc


def _prep_shard(Xc):
    # (512,256) -> (128, 4*256), row (c*128+p) -> [p, c*256:(c+1)*256]
    return np.ascontiguousarray(
        Xc.reshape(C_SLOTS, 128, D).transpose(1, 0, 2).reshape(128, FREE))


def kernel(X, Y, _debug=False):
    from concourse import bass_utils

    X = np.asarray(X, dtype=np.float32)
    nc = _build_nc(debug=_debug)
    in_maps = [{"x": _prep_shard(X[c * ROWS_PER_CORE:(c + 1) * ROWS_PER_CORE])}
               for c in range(N_CORES)]
    res = bass_utils.run_bass_kernel_spmd(nc, in_maps, list(range(N_CORES)))
    total = np.float64(0.0)
    for c in range(N_CORES):
        total += res.results[c]["partial"].astype(np.float64).sum()
    out = np.array([total], dtype=np.float32)
    if _debug:
        return out, res
    return out


# revision 11
# speedup vs baseline: 1.5521x; 1.5521x over previous
import numpy as np

N_ROWS = 4096
D = 256
N_CORES = 8
ROWS_PER_CORE = N_ROWS // N_CORES  # 512
C_SLOTS = ROWS_PER_CORE // 128     # 4
FREE = C_SLOTS * D                 # 1024
HALF = FREE // 2                   # 512 (c-slots 0-1 vs 2-3)


def _build_nc(debug=False):
    from concourse import bass
    from concourse import mybir
    from contextlib import ExitStack

    f32 = mybir.dt.float32
    AO = mybir.AluOpType
    nc = bass.Bass()
    x_ext = nc.declare_dram_parameter("x", [128, FREE], f32, isOutput=False)
    m256_ext = nc.declare_dram_parameter("m256", [128, FREE], f32, isOutput=False)
    m32_ext = nc.declare_dram_parameter("m32", [128, FREE], f32, isOutput=False)
    m8_ext = nc.declare_dram_parameter("m8", [128, 16], f32, isOutput=False)
    part_ext = nc.declare_dram_parameter("partial", [128, 1], f32, isOutput=True)
    dbg_ext = {}
    if debug:
        for nm in ["s_out", "g_out", "d2_out", "dact", "dnew"]:
            dbg_ext[nm] = nc.declare_dram_parameter(nm, [128, C_SLOTS], f32, isOutput=True)

    with ExitStack() as ctx:
        dma_sem = ctx.enter_context(nc.semaphore("dma_sem"))
        v_sem = ctx.enter_context(nc.semaphore("v_sem"))
        a_sem = ctx.enter_context(nc.semaphore("a_sem"))

        def sb(name, shape):
            return ctx.enter_context(nc.sbuf_tensor(name, shape, f32))

        x_sb = sb("x_sb", [128, FREE])
        m256_sb = sb("m256_sb", [128, FREE])
        m32_sb = sb("m32_sb", [128, FREE])
        m8_sb = sb("m8_sb", [128, 16])
        sq = sb("sq_sb", [128, FREE])
        lo = sb("lo_sb", [128, FREE])
        P = sb("p_scan", [128, FREE])
        W = sb("w_scan", [128, FREE])
        cbuf = sb("c_sb", [128, FREE])
        t1b = sb("t1b", [128, FREE])
        t2b = sb("t2b", [128, FREE])
        t3b = sb("t3b", [128, FREE])
        wext = sb("wext", [128, 32])
        comb = sb("comb", [128, 32])
        ssum = sb("ssum", [128, C_SLOTS])
        Ab = sb("Ab", [128, C_SLOTS])
        Db = sb("Db", [128, C_SLOTS])
        ub = sb("ub", [128, C_SLOTS])
        d2c = sb("d2c", [128, C_SLOTS])
        n1 = sb("n1", [128, C_SLOTS])
        n2 = sb("n2", [128, C_SLOTS])
        n3 = sb("n3", [128, C_SLOTS])
        dact_sb = sb("dact_sb", [128, C_SLOTS])
        dnew_sb = sb("dnew_sb", [128, C_SLOTS])
        part_sb = sb("part_sb", [128, 1])
        block = ctx.enter_context(nc.Block())

        @block.sync
        def _(sync):
            sync.dma_start(out=x_sb[:], in_=x_ext[:]).then_inc(dma_sem, 16)
            sync.dma_start(out=m256_sb[:], in_=m256_ext[:]).then_inc(dma_sem, 16)
            sync.dma_start(out=m32_sb[:], in_=m32_ext[:]).then_inc(dma_sem, 16)
            sync.dma_start(out=m8_sb[:], in_=m8_ext[:]).then_inc(dma_sem, 16)
            sync.wait_ge(v_sem, 2)
            sync.dma_start(out=part_ext[:], in_=part_sb[:]).then_inc(dma_sem, 16)
            n_out = 5
            if debug:
                for nm, src in [("s_out", ssum), ("g_out", Ab), ("d2_out", d2c),
                                ("dact", dact_sb), ("dnew", dnew_sb)]:
                    sync.dma_start(out=dbg_ext[nm][:], in_=src[:]).then_inc(dma_sem, 16)
                    n_out += 1
            sync.wait_ge(dma_sem, 16 * n_out)

        @block.vector
        def _(v):
            v.wait_ge(dma_sem, 16)

            # Two independent half-programs (c-slots 0-1 vs 2-3) with strictly
            # alternating instructions: a DVE write is not visible to the
            # immediately-next DVE instruction, but is at distance >= 2.
            def prog(h):
                fb = h * HALF
                cb = h * (C_SLOTS // 2)
                nsl = C_SLOTS // 2

                def big(t):
                    return t[:, fb:fb + HALF]

                def small(t):
                    return t[:, cb:cb + nsl]

                # hi_k = fl(x^2); Dekker lo_k = x^2 - fl(x^2) exactly
                yield v.tensor_mul(big(sq), big(x_sb), big(x_sb))
                yield v.tensor_scalar_mul(big(t1b), big(x_sb), 4097.0)
                yield v.tensor_sub(big(t2b), big(t1b), big(x_sb))
                yield v.tensor_sub(big(t1b), big(t1b), big(t2b))     # xh
                yield v.tensor_sub(big(t2b), big(x_sb), big(t1b))    # xl
                yield v.tensor_mul(big(t3b), big(t1b), big(t1b))     # xh*xh
                yield v.tensor_sub(big(t3b), big(t3b), big(sq))
                yield v.tensor_mul(big(t1b), big(t1b), big(t2b))     # xh*xl
                yield v.tensor_add(big(t1b), big(t1b), big(t1b))     # 2*xh*xl
                yield v.tensor_add(big(t3b), big(t3b), big(t1b))
                yield v.tensor_mul(big(t2b), big(t2b), big(t2b))     # xl*xl
                yield v.tensor_add(big(lo), big(t3b), big(t2b))

                # plain RN prefix chain of hi_k per 256-chunk (mask-reset scan)
                yield v.wait_ge(dma_sem, 64)
                yield v.tensor_tensor_scan(big(P), big(m256_sb), big(sq), 0.0,
                                           AO.mult, AO.add)

                # s_i: 8 windows of 32 sequential adds (mask32 scan), then
                # sequential combine of the 8 window sums (mask8 scan)
                yield v.tensor_tensor_scan(big(W), big(m32_sb), big(sq), 0.0,
                                           AO.mult, AO.add)
                wv = wext[:, 16 * h:16 * h + 16]
                yield v.tensor_copy(wv, W[:, fb + 31:fb + HALF:32])
                cv = comb[:, 16 * h:16 * h + 16]
                yield v.tensor_tensor_scan(cv, m8_sb[:], wv, 0.0, AO.mult, AO.add)
                yield v.tensor_copy(small(ssum), comb[:, 16 * h + 7:16 * h + 16:8])

                # vectorized Knuth TwoSum: exact per-step error e_k of the
                # P-scan adds, then c_k = RN(e_k + lo_k)
                WW = HALF - 1
                cp_P = P[:, fb + 1:fb + HALF]
                pv_P = P[:, fb:fb + WW]
                cp_h = sq[:, fb + 1:fb + HALF]
                cp_l = lo[:, fb + 1:fb + HALF]
                b1 = t1b[:, fb:fb + WW]
                b2 = t2b[:, fb:fb + WW]
                b3 = t3b[:, fb:fb + WW]
                yield v.tensor_sub(b1, cp_P, pv_P)                   # b
                yield v.tensor_sub(b2, cp_P, b1)                     # t1
                yield v.tensor_sub(b2, pv_P, b2)                     # e1
                yield v.tensor_sub(b3, cp_h, b1)                     # e2
                yield v.tensor_add(b2, b2, b3)                       # e
                yield v.tensor_add(cbuf[:, fb + 1:fb + HALF], b2, cp_l)

                # sequential correction: A_k = RN(P_k + RN(c_k + D_{k-1})),
                # D_k = RN(A_k - P_k);  A_255 = true FMA-chain value
                def kview(t, k):
                    return t[:, fb + k:fb + HALF:D]  # (128,2)

                Av, Dv, uv = small(Ab), small(Db), small(ub)
                yield v.tensor_copy(Av, kview(P, 0))
                yield v.tensor_scalar_mul(Dv, kview(P, 0), 0.0)
                for k in range(1, D):
                    yield v.tensor_add(uv, kview(cbuf, k), Dv)
                    yield v.tensor_add(Av, uv, kview(P, k))
                    yield v.tensor_sub(Dv, Av, kview(P, k))

                # d2 = fl(2s - 2g), clip at 0
                sv = small(ssum)
                yield v.tensor_add(small(n1), sv, sv)
                yield v.tensor_add(small(n2), Av, Av)
                yield v.tensor_sub(small(n3), small(n1), small(n2))
                yield v.tensor_scalar_max(small(d2c), small(n3), 0.0)

            g0, g1 = prog(0), prog(1)
            while True:
                done = 0
                for g in (g0, g1):
                    try:
                        next(g)
                    except StopIteration:
                        done += 1
                if done == 2:
                    break
            # spacer guaranteeing d2c committed before cross-engine read
            v.tensor_copy(t1b[:, 0:HALF], sq[:, 0:HALF]).then_inc(v_sem)

            # ---- Newton refine: d = 0.5*(y + d2 * recip(y)) ----
            v.wait_ge(a_sem, 1)

            def newton(h):
                cb = h * (C_SLOTS // 2)
                nsl = C_SLOTS // 2

                def small(t):
                    return t[:, cb:cb + nsl]

                yield v.tensor_scalar_max(small(n1), small(dact_sb), 1e-30)
                yield v.reciprocal(small(n2), small(n1))
                yield v.tensor_mul(small(n2), small(n2), small(d2c))
                yield v.tensor_add(small(n3), small(dact_sb), small(n2))
                yield v.tensor_scalar_mul(small(dnew_sb), small(n3), 0.5)

            g0, g1 = newton(0), newton(1)
            while True:
                done = 0
                for g in (g0, g1):
                    try:
                        next(g)
                    except StopIteration:
                        done += 1
                if done == 2:
                    break
            # spacer before reading dnew in the reduce
            v.tensor_copy(t2b[:, 0:HALF], sq[:, 0:HALF])
            v.tensor_reduce(part_sb[:], dnew_sb[:], mybir.AxisListType.X,
                            mybir.AluOpType.add).then_inc(v_sem)

        @block.scalar
        def _(s):
            s.wait_ge(v_sem, 1)
            s.sqrt(dact_sb[:], d2c[:]).then_inc(a_sem)

    return nc


def _prep_shard(Xc):
    # (512,256) -> (128, 4*256), row (c*128+p) -> [p, c*256:(c+1)*256]
    return np.ascontiguousarray(
        Xc.reshape(C_SLOTS, 128, D).transpose(1, 0, 2).reshape(128, FREE))


def _masks():
    m256 = np.ones((128, FREE), np.float32)
    m256[:, 0::D] = 0.0
    m32 = np.ones((128, FREE), np.float32)
    m32[:, 0::32] = 0.0
    m8 = np.ones((128, 16), np.float32)
    m8[:, 0::8] = 0.0
    return m256, m32, m8


def _in_maps(X):
    m256, m32, m8 = _masks()
    return [{"x": _prep_shard(X[c * ROWS_PER_CORE:(c + 1) * ROWS_PER_CORE]),
             "m256": m256, "m32": m32, "m8": m8} for c in range(N_CORES)]


def kernel(X, Y, _debug=False):
    from concourse import bass_utils

    X = np.asarray(X, dtype=np.float32)
    nc = _build_nc(debug=_debug)
    in_maps = _in_maps(X)
    res = None
    for attempt in range(3):
        try:
            res = bass_utils.run_bass_kernel_spmd(nc, in_maps, list(range(N_CORES)))
            break
        except Exception:
            if attempt == 2:
                raise
    total = np.float64(0.0)
    for c in range(N_CORES):
        total += res.results[c]["partial"].astype(np.float64).sum()
    out = np.array([total], dtype=np.float32)
    if _debug:
        return out, res
    return out


# revision 18
# speedup vs baseline: 1.5581x; 1.0039x over previous
import numpy as np

N_ROWS = 4096
D = 256
N_CORES = 8
ROWS_PER_CORE = N_ROWS // N_CORES  # 512
C_SLOTS = ROWS_PER_CORE // 128     # 4
FREE = C_SLOTS * D                 # 1024
HALF = FREE // 2                   # 512 (c-slots 0-1 vs 2-3)


def _build_nc(debug=False):
    from concourse import bass
    from concourse import mybir
    from contextlib import ExitStack

    f32 = mybir.dt.float32
    AO = mybir.AluOpType
    nc = bass.Bass()
    x_ext = nc.declare_dram_parameter("x", [128, FREE], f32, isOutput=False)
    m256_ext = nc.declare_dram_parameter("m256", [128, FREE], f32, isOutput=False)
    m32_ext = nc.declare_dram_parameter("m32", [128, FREE], f32, isOutput=False)
    m8_ext = nc.declare_dram_parameter("m8", [128, 16], f32, isOutput=False)
    part_ext = nc.declare_dram_parameter("partial", [128, 1], f32, isOutput=True)
    dbg_ext = {}
    if debug:
        for nm in ["s_out", "g_out", "d2_out", "dact", "dnew"]:
            dbg_ext[nm] = nc.declare_dram_parameter(nm, [128, C_SLOTS], f32, isOutput=True)

    with ExitStack() as ctx:
        dma_sem = ctx.enter_context(nc.semaphore("dma_sem"))
        v_sem = ctx.enter_context(nc.semaphore("v_sem"))
        a_sem = ctx.enter_context(nc.semaphore("a_sem"))

        def sb(name, shape):
            return ctx.enter_context(nc.sbuf_tensor(name, shape, f32))

        x_sb = sb("x_sb", [128, FREE])
        m256_sb = sb("m256_sb", [128, FREE])
        m32_sb = sb("m32_sb", [128, FREE])
        m8_sb = sb("m8_sb", [128, 16])
        sq = sb("sq_sb", [128, FREE])
        lo = sb("lo_sb", [128, FREE])
        P = sb("p_scan", [128, FREE])
        W = sb("w_scan", [128, FREE])
        cbuf = sb("c_sb", [128, FREE])
        t1b = sb("t1b", [128, FREE])
        t2b = sb("t2b", [128, FREE])
        t3b = sb("t3b", [128, FREE])
        wext = sb("wext", [128, 32])
        comb = sb("comb", [128, 32])
        ssum = sb("ssum", [128, C_SLOTS])
        Ab = sb("Ab", [128, C_SLOTS])
        Db = sb("Db", [128, C_SLOTS])
        ub = sb("ub", [128, C_SLOTS])
        d2c = sb("d2c", [128, C_SLOTS])
        n1 = sb("n1", [128, C_SLOTS])
        n2 = sb("n2", [128, C_SLOTS])
        n3 = sb("n3", [128, C_SLOTS])
        dact_sb = sb("dact_sb", [128, C_SLOTS])
        dnew_sb = sb("dnew_sb", [128, C_SLOTS])
        part_sb = sb("part_sb", [128, 1])
        block = ctx.enter_context(nc.Block())

        @block.sync
        def _(sync):
            sync.dma_start(out=x_sb[:, 0:HALF], in_=x_ext[:, 0:HALF]).then_inc(dma_sem, 16)
            sync.dma_start(out=x_sb[:, HALF:FREE], in_=x_ext[:, HALF:FREE]).then_inc(dma_sem, 16)
            sync.dma_start(out=m256_sb[:], in_=m256_ext[:]).then_inc(dma_sem, 16)
            sync.dma_start(out=m32_sb[:], in_=m32_ext[:]).then_inc(dma_sem, 16)
            sync.dma_start(out=m8_sb[:], in_=m8_ext[:]).then_inc(dma_sem, 16)
            sync.wait_ge(v_sem, 2)
            sync.dma_start(out=part_ext[:], in_=part_sb[:]).then_inc(dma_sem, 16)
            n_out = 6
            if debug:
                for nm, src in [("s_out", ssum), ("g_out", Ab), ("d2_out", d2c),
                                ("dact", dact_sb), ("dnew", dnew_sb)]:
                    sync.dma_start(out=dbg_ext[nm][:], in_=src[:]).then_inc(dma_sem, 16)
                    n_out += 1
            sync.wait_ge(dma_sem, 16 * n_out)

        @block.vector
        def _(v):
            v.wait_ge(dma_sem, 16)

            # Two independent half-programs (c-slots 0-1 vs 2-3) with strictly
            # alternating instructions: a DVE write is not visible to the
            # immediately-next DVE instruction, but is at distance >= 2.
            def prog(h):
                fb = h * HALF
                cb = h * (C_SLOTS // 2)
                nsl = C_SLOTS // 2

                def big(t):
                    return t[:, fb:fb + HALF]

                def small(t):
                    return t[:, cb:cb + nsl]

                # hi_k = fl(x^2); Dekker lo_k = x^2 - fl(x^2) exactly
                yield v.wait_ge(dma_sem, 32 if h else 16)
                yield v.tensor_mul(big(sq), big(x_sb), big(x_sb))
                yield v.tensor_scalar_mul(big(t1b), big(x_sb), 4097.0)
                yield v.tensor_sub(big(t2b), big(t1b), big(x_sb))
                yield v.tensor_sub(big(t1b), big(t1b), big(t2b))     # xh
                yield v.tensor_sub(big(t2b), big(x_sb), big(t1b))    # xl
                yield v.tensor_mul(big(t3b), big(t1b), big(t1b))     # xh*xh
                yield v.tensor_sub(big(t3b), big(t3b), big(sq))
                yield v.tensor_mul(big(t1b), big(t1b), big(t2b))     # xh*xl
                yield v.tensor_add(big(t1b), big(t1b), big(t1b))     # 2*xh*xl
                yield v.tensor_add(big(t3b), big(t3b), big(t1b))
                yield v.tensor_mul(big(t2b), big(t2b), big(t2b))     # xl*xl
                yield v.tensor_add(big(lo), big(t3b), big(t2b))

                # plain RN prefix chain of hi_k per 256-chunk (mask-reset scan)
                yield v.wait_ge(dma_sem, 80)
                yield v.tensor_tensor_scan(big(P), big(m256_sb), big(sq), 0.0,
                                           AO.mult, AO.add)

                # s_i: 8 windows of 32 sequential adds (mask32 scan), then
                # sequential combine of the 8 window sums (mask8 scan)
                yield v.tensor_tensor_scan(big(W), big(m32_sb), big(sq), 0.0,
                                           AO.mult, AO.add)
                wv = wext[:, 16 * h:16 * h + 16]
                yield v.tensor_copy(wv, W[:, fb + 31:fb + HALF:32])
                cv = comb[:, 16 * h:16 * h + 16]
                yield v.tensor_tensor_scan(cv, m8_sb[:], wv, 0.0, AO.mult, AO.add)
                yield v.tensor_copy(small(ssum), comb[:, 16 * h + 7:16 * h + 16:8])

                # exact per-step error e_k of the P-scan adds, then
                # c_k = RN(e_k + lo_k).  Fast2Sum full-width (valid once the
                # running sum dominates, k >= 48), Knuth TwoSum patch for k<48.
                WW = HALF - 1
                cp_P = P[:, fb + 1:fb + HALF]
                pv_P = P[:, fb:fb + WW]
                cp_h = sq[:, fb + 1:fb + HALF]
                cp_l = lo[:, fb + 1:fb + HALF]
                b1 = t1b[:, fb:fb + WW]
                b2 = t2b[:, fb:fb + WW]
                yield v.tensor_sub(b1, cp_P, pv_P)                   # z
                yield v.tensor_sub(b2, cp_h, b1)                     # e (Fast2Sum)
                for s in range(nsl):
                    o = fb + s * D
                    KW = 47
                    cpP_s = P[:, o + 1:o + 1 + KW]
                    pvP_s = P[:, o:o + KW]
                    cph_s = sq[:, o + 1:o + 1 + KW]
                    k1 = t3b[:, fb + 128 * s:fb + 128 * s + KW]
                    k3 = t3b[:, fb + 128 * s + 64:fb + 128 * s + 64 + KW]
                    e_s = t2b[:, fb + s * D:fb + s * D + KW]
                    yield v.tensor_sub(k1, cpP_s, pvP_s)             # b
                    yield v.tensor_sub(e_s, cpP_s, k1)               # t1
                    yield v.tensor_sub(e_s, pvP_s, e_s)              # e1
                    yield v.tensor_sub(k3, cph_s, k1)                # e2
                    yield v.tensor_add(e_s, e_s, k3)                 # e (Knuth)
                yield v.tensor_add(cbuf[:, fb + 1:fb + HALF], b2, cp_l)

                # sequential correction: A_k = RN(P_k + RN(c_k + D_{k-1})),
                # D_k = RN(A_k - P_k);  A_255 = true FMA-chain value
                def kview(t, k):
                    return t[:, fb + k:fb + HALF:D]  # (128,2)

                Av, Dv, uv = small(Ab), small(Db), small(ub)
                yield v.tensor_copy(Av, kview(P, 0))
                yield v.tensor_scalar_mul(Dv, kview(P, 0), 0.0)
                for k in range(1, D):
                    yield v.tensor_add(uv, kview(cbuf, k), Dv)
                    yield v.tensor_add(Av, uv, kview(P, k))
                    yield v.tensor_sub(Dv, Av, kview(P, k))

                # d2 = fl(2s - 2g), clip at 0
                sv = small(ssum)
                yield v.tensor_add(small(n1), sv, sv)
                yield v.tensor_add(small(n2), Av, Av)
                yield v.tensor_sub(small(n3), small(n1), small(n2))
                yield v.tensor_scalar_max(small(d2c), small(n3), 0.0)

            g0, g1 = prog(0), prog(1)
            while True:
                done = 0
                for g in (g0, g1):
                    try:
                        next(g)
                    except StopIteration:
                        done += 1
                if done == 2:
                    break
            # spacer guaranteeing d2c committed before cross-engine read
            v.tensor_copy(t1b[:, 0:64], sq[:, 0:64]).then_inc(v_sem)

            v.wait_ge(a_sem, 1)
            v.tensor_copy(dnew_sb[:], dact_sb[:])
            v.tensor_reduce(part_sb[:], dact_sb[:], mybir.AxisListType.X,
                            mybir.AluOpType.add).then_inc(v_sem)

        @block.scalar
        def _(s):
            s.wait_ge(v_sem, 1)
            s.sqrt(dact_sb[:], d2c[:]).then_inc(a_sem)

    return nc


def _prep_shard(Xc):
    # (512,256) -> (128, 4*256), row (c*128+p) -> [p, c*256:(c+1)*256]
    return np.ascontiguousarray(
        Xc.reshape(C_SLOTS, 128, D).transpose(1, 0, 2).reshape(128, FREE))


def _masks():
    m256 = np.ones((128, FREE), np.float32)
    m256[:, 0::D] = 0.0
    m32 = np.ones((128, FREE), np.float32)
    m32[:, 0::32] = 0.0
    m8 = np.ones((128, 16), np.float32)
    m8[:, 0::8] = 0.0
    return m256, m32, m8


def _in_maps(X):
    m256, m32, m8 = _masks()
    return [{"x": _prep_shard(X[c * ROWS_PER_CORE:(c + 1) * ROWS_PER_CORE]),
             "m256": m256, "m32": m32, "m8": m8} for c in range(N_CORES)]


def kernel(X, Y, _debug=False):
    from concourse import bass_utils

    X = np.asarray(X, dtype=np.float32)
    nc = _build_nc(debug=_debug)
    in_maps = _in_maps(X)
    res = None
    for attempt in range(3):
        try:
            res = bass_utils.run_bass_kernel_spmd(nc, in_maps, list(range(N_CORES)))
            break
        except Exception:
            if attempt == 2:
                raise
    total = np.float64(0.0)
    for c in range(N_CORES):
        total += res.results[c]["partial"].astype(np.float64).sum()
    out = np.array([total], dtype=np.float32)
    if _debug:
        return out, res
    return out
